# revision 44
# baseline (speedup 1.0000x reference)
"""MiniBatchDiscrimination kernel for 8 Trainium2 NeuronCores.

Math: m = (x @ T).reshape(B, K, D); l1[i,k,j] = sum_d |m[i,k,d]-m[j,k,d]|;
feat[i,k] = sum_j exp(-l1[i,k,j]); out = concat([x, feat], axis=1).

Strategy (per core, fully SPMD-uniform):
- Layout [kd, j]: kd = 5k+d rows on partitions (4 sets of 125), j in the
  free dim. The DVE has no single-op |x-s| on trn2, so use
  |y| = 2*relu(y) - y:  l1 = 2*sum_d relu(diff_d) - (Msum_j - Msum_i)
  with Msum = sum_d m_d precomputed (host passes T5 = per-k summed T).
  One 4x-mode bf16 tensor_scalar(sub -> max 0) per (i, set) makes the
  relu slab; PE reduces d via bf16 selector matmuls (weight 2.0) plus a
  rank-25 -Msum correction matmul; the +Msum_i term rides the Exp bias.
- ScalarE Exp(scale=-1, bias=-Msum_i, accum_out) fuses exp with the
  j-reduction, packing 5 consecutive i into one [125, jw] PSUM tile.
- Pair symmetry: each unordered pair {i, j} is computed once. Inputs are
  row-rotated on host so each core's own block sits at j=0; core half is
  distinguished by rotating T's columns by 250 for cores 4-7, making the
  window widths per k-set uniform: sets 0,1 see 5 j-blocks (640), sets
  2,3 see 4 (512). Column-sum partials (feat[j] contributions) come from
  small PE matmuls over the exp output, excluding the own block; the
  host adds row-window and column partials during the gather.
"""

import numpy as np
from contextlib import ExitStack

import ml_dtypes
import concourse.bass as bass
import concourse.tile as tile
from concourse import mybir
from concourse.bass_utils import run_bass_kernel_spmd

B, F = 1024, 1024
K, D = 100, 5
KD = K * D            # 500
NCORES = 8
RPC = B // NCORES     # 128 i-rows per core
SETK = 25             # kernels per set
NSET = 4
SROWS = SETK * D      # 125 kd rows per set
GSZ = 5               # i per group (packs 5x25 k into 125 psum rows)
NG = (RPC + GSZ - 1) // GSZ    # 26 (last group has 3)
JW = [640, 640, 512, 512]      # per-set j-window width (own block first)
CSW = 512             # colsum output width (j columns 128..640)
NCOL = NG * NSET      # 104 feat5 columns
FP32 = mybir.dt.float32
BF16 = mybir.dt.float16
AF = mybir.ActivationFunctionType
ALU = mybir.AluOpType
NPBF = np.float16


class TC(tile.TileContext):
    """TileContext whose tail puts sem waits on NOPs instead of the Drain.

    The walrus in this container lowers Drain/NOP with a no-sync-struct ISA
    encoding that holds at most one wait, so the stock tail drain (which
    carries one wait per outstanding proc) fails codegen. Emit one NOP per
    proc, each carrying a single wait, before the drain.
    """

    def _drain_and_barrier(self, tick_clock, wait_clock):
        from concourse.vector_clock import ScopedClock, VectorClock

        gc = tick_clock.global_clock
        n = len(gc)
        for p in range(n):
            t = gc[p]
            if t <= 0:
                continue
            vec = [0] * n
            vec[p] = t
            nop_inst = self.nc.sync.nop(nofuse=True)
            wait_clock.add_sem_waits(
                nop_inst.ins, ScopedClock({None: VectorClock(vec)})
            )
        self.nc.sync.drain()
        self.nc.all_engine_barrier()
        popped = self.nc._tile_sem_poison_stack.pop()
        assert popped is self._sem_poison
        self.nc.clear_and_free_semaphores(list(self.sems.allocated().values()))
        self.nc.all_engine_barrier()


def _hoist_excess_waits(nc):
    """Move excess sem waits onto same-engine NOPs inserted just before.

    This container's walrus encodes Matmult (LDWEIGHTS struct) and
    NoOp/Drain with room for a single sync wait; Tile may attach several.
    Keep one wait on the instruction and carry the rest on dedicated NOPs,
    which is sync-equivalent (same engine, program order).
    """
    def limit_for(inst):
        return 1
    for f in nc.m.functions:
        for bb in f.blocks:
            snapshot = list(bb.instructions)
            if not any(
                i.sync_info is not None
                and len(i.sync_info.on_wait) > limit_for(i)
                for i in snapshot
            ):
                continue
            new_list = []
            for inst in snapshot:
                lim = limit_for(inst)
                si = inst.sync_info
                if lim is not None and si is not None and \
                        len(si.on_wait) > lim:
                    waits = list(si.on_wait)
                    for w in waits[:-lim]:
                        bi = nc.engines[inst.engine].nop(nofuse=True)
                        found = False
                        for f2 in nc.m.functions:
                            for bb2 in f2.blocks:
                                tail = bb2.instructions
                                if tail and tail[-1].name == bi.ins.name:
                                    tail.pop()
                                    found = True
                                    break
                            if found:
                                break
                        assert found, bi.ins.name
                        bi.ins.sync_info = mybir.SyncInfo(
                            on_wait=[w], on_update=[])
                        new_list.append(bi.ins)
                    inst.sync_info = mybir.SyncInfo(
                        on_wait=waits[-lim:], on_update=list(si.on_update))
                new_list.append(inst)
            bb.instructions = new_list


def _sel_arrays():
    """Constant selector matrices, passed in as inputs (bf16).

    selr[r][c, m] = 2 iff m == 25*r + c//5   (2*relu d-reduction, [125,125])
    negc[v][c, m] = -1 iff m == 25*r + c, r < nr(v)   (-Msum correction,
      [25, 125]; v=0: all 5 r-blocks, v=1: first 3 for the last group)
    selc[s][c, m] = 1 iff m == 25*s + c%25   (colsum, [125, 100]);
      the second 4 (for the 3-row last group) zero rows c >= 75.
    """
    selr = np.zeros((GSZ, SROWS, SROWS), np.float32)
    c = np.arange(SROWS)
    for r in range(GSZ):
        selr[r, c, SETK * r + c // D] = 2.0
    negc = np.zeros((2, SETK, SROWS), np.float32)
    ck = np.arange(SETK)
    for r in range(GSZ):
        negc[0, ck, SETK * r + ck] = -1.0
        if r < 3:
            negc[1, ck, SETK * r + ck] = -1.0
    selc = np.zeros((2 * NSET, SROWS, K), np.float32)
    for s in range(NSET):
        selc[s, c, SETK * s + c % SETK] = 1.0
        selc[NSET + s] = selc[s]
        selc[NSET + s, 3 * SETK:, :] = 0.0
    return selr.astype(NPBF), negc.astype(NPBF), selc.astype(NPBF)


def _chunks(jw):
    return [(c0, min(512, jw - c0)) for c0 in range(0, jw, 512)]


def build_nc(reps: int = 1):
    nc = bass.Bass()
    xt_d = nc.dram_tensor("xt", [F, B], BF16, kind="ExternalInput")
    t_d = nc.dram_tensor("t", [F, KD], BF16, kind="ExternalInput")
    t5_d = nc.dram_tensor("t5", [F, K], BF16, kind="ExternalInput")
    selr_d = nc.dram_tensor("selr", [GSZ * SROWS, SROWS], BF16,
                            kind="ExternalInput")
    negc_d = nc.dram_tensor("negc", [2 * SETK, SROWS], BF16,
                            kind="ExternalInput")
    selc_d = nc.dram_tensor("selc", [2 * NSET * SROWS, K], BF16,
                            kind="ExternalInput")
    feat5_d = nc.dram_tensor("feat5", [SROWS, NCOL], FP32,
                             kind="ExternalOutput")
    colt_d = nc.dram_tensor("colt", [K, CSW], FP32, kind="ExternalOutput")

    with TC(nc) as tc, ExitStack() as ctx:
        const = ctx.enter_context(tc.tile_pool(name="const", bufs=1))

        M = [const.tile([SROWS, JW[s]], BF16, tag=f"M{s}", name=f"M{s}")
             for s in range(NSET)]
        S = [const.tile([SROWS, RPC], FP32, tag=f"S{s}", name=f"S{s}")
             for s in range(NSET)]
        Msum = const.tile([K, max(JW)], BF16, tag="Msum")
        Msum_s = [const.tile([SETK, JW[s]], BF16, tag=f"Msum{s}",
                             name=f"Msum{s}") for s in range(NSET)]
        Msum32 = const.tile([K, RPC], FP32, tag="Msum32")
        biasn = const.tile([SROWS, NCOL], FP32, tag="biasn")
        feat5 = const.tile([SROWS, NCOL], FP32, tag="feat5")
        selr_t = [const.tile([SROWS, SROWS], BF16, tag=f"selr{r}",
                             name=f"selr{r}")
                  for r in range(GSZ)]
        negc_t = [const.tile([SETK, SROWS], BF16, tag=f"negc{v}",
                             name=f"negc{v}")
                  for v in range(2)]
        selc_t = [const.tile([SROWS, K], BF16, tag=f"selc{v}",
                             name=f"selc{v}")
                  for v in range(2 * NSET)]
        for r in range(GSZ):
            nc.gpsimd.dma_start(selr_t[r][:],
                                selr_d[r * SROWS:(r + 1) * SROWS, :])
        for v in range(2):
            nc.gpsimd.dma_start(negc_t[v][:],
                                negc_d[v * SETK:(v + 1) * SETK, :])
        for v in range(2 * NSET):
            nc.gpsimd.dma_start(selc_t[v][:],
                                selc_d[v * SROWS:(v + 1) * SROWS, :])

        # ---- projection: M[s] = bf16((x @ T).T slab), Msum = (x @ T5).T --
        with ExitStack() as sctx:
            spool = sctx.enter_context(tc.tile_pool(name="setup", bufs=1))
            ppsum = sctx.enter_context(
                tc.tile_pool(name="ppsum", bufs=4, space="PSUM"))
            xt = [spool.tile([128, B], BF16, tag=f"xt{fb}", name=f"xt{fb}")
                  for fb in range(8)]
            tT = [spool.tile([128, KD], BF16, tag=f"tT{fb}", name=f"tT{fb}")
                  for fb in range(8)]
            t5T = [spool.tile([128, K], BF16, tag=f"t5T{fb}",
                              name=f"t5T{fb}") for fb in range(8)]
            for fb in range(8):
                nc.sync.dma_start(t5T[fb][:],
                                  t5_d[fb * 128:(fb + 1) * 128, :])
                nc.sync.dma_start(xt[fb][:],
                                  xt_d[fb * 128:(fb + 1) * 128, :])
                nc.sync.dma_start(tT[fb][:],
                                  t_d[fb * 128:(fb + 1) * 128, :])
            # Msum first: the bias table gates the first Exp
            for c0, cw in _chunks(max(JW)):
                ps5 = ppsum.tile([K, 512], FP32, tag="pps5")
                for fb in range(8):
                    nc.tensor.matmul(
                        ps5[:, 0:cw], t5T[fb][:], xt[fb][:, c0:c0 + cw],
                        start=(fb == 0), stop=(fb == 7))
                nc.scalar.copy(Msum[:, c0:c0 + cw], ps5[:, 0:cw])
            nc.scalar.activation(Msum32[:], Msum[:, 0:RPC], AF.Copy,
                                 scale=-1.0)
            # per-set correction operands at partition base 0 (PE rhs rule)
            for s in range(NSET):
                nc.gpsimd.dma_start(
                    Msum_s[s][:], Msum[s * SETK:(s + 1) * SETK, 0:JW[s]])
            # bias table: biasn[25r + kk, 4g + s] = -Msum[25s + kk, 5g + r]
            # (Msum32 already negated); DMAs split across the SP and Pool
            # queues so the per-column subtile deps release exps early
            nc.gpsimd.memset(biasn[:], 0.0)
            for r in range(GSZ):
                ng = NG if r < 3 else NG - 1
                for s in range(NSET):
                    q = nc.sync if (r + s) % 2 == 0 else nc.gpsimd
                    q.dma_start(
                        biasn[SETK * r:SETK * r + SETK,
                              s:s + 4 * (ng - 1) + 1:4],
                        Msum32[SETK * s:SETK * s + SETK,
                               r:r + 5 * (ng - 1) + 1:5])
            for s in range(NSET):
                kd0 = s * SROWS
                for ci, (c0, cw) in enumerate(_chunks(JW[s])):
                    ps = ppsum.tile([SROWS, 512], FP32, tag="pps")
                    for fb in range(8):
                        nc.tensor.matmul(
                            ps[:, 0:cw],
                            tT[fb][:, kd0:kd0 + SROWS],
                            xt[fb][:, c0:c0 + cw],
                            start=(fb == 0), stop=(fb == 7))
                    if (s + ci) % 2 == 0:
                        nc.vector.tensor_copy(M[s][:, c0:c0 + cw],
                                              ps[:, 0:cw])
                    else:
                        nc.scalar.copy(M[s][:, c0:c0 + cw], ps[:, 0:cw])
                nc.scalar.activation(S[s][:], M[s][:, 0:RPC], AF.Copy)

        # ---- main loop ----
        absd_pool = ctx.enter_context(tc.tile_pool(name="absd", bufs=6))
        e_pool = ctx.enter_context(tc.tile_pool(name="E", bufs=2))
        l1_640 = ctx.enter_context(
            tc.tile_pool(name="l1a", bufs=2, space="PSUM"))
        l1_512 = ctx.enter_context(
            tc.tile_pool(name="l1b", bufs=3, space="PSUM"))
        cs_pool = ctx.enter_context(
            tc.tile_pool(name="cs", bufs=1, space="PSUM"))
        cspsum = cs_pool.tile([K, CSW], FP32, tag="cs")

        for rep in range(reps):
            for g in range(NG):
                nr = min(GSZ, RPC - GSZ * g)
                for s in range(NSET):
                    jw = JW[s]
                    l1p = l1_640 if jw == 640 else l1_512
                    l1 = l1p.tile([SROWS, jw], FP32, tag=f"l1{jw}")
                    absd = [absd_pool.tile([SROWS, jw], BF16,
                                           tag=f"a{jw}_{r}", name=f"a{r}")
                            for r in range(nr)]
                    for r in range(nr):
                        i = GSZ * g + r
                        nc.vector.tensor_scalar(
                            absd[r][:], M[s][:, 0:jw], S[s][:, i:i + 1],
                            0.0, op0=ALU.subtract, op1=ALU.max)
                    for c0, cw in _chunks(jw):
                        for r in range(nr):
                            nc.tensor.matmul(
                                l1[:, c0:c0 + cw], selr_t[r][:],
                                absd[r][:, c0:c0 + cw],
                                start=(r == 0), stop=False)
                        nc.tensor.matmul(
                            l1[:, c0:c0 + cw],
                            negc_t[0 if nr == GSZ else 1][:],
                            Msum_s[s][:, c0:c0 + cw],
                            start=False, stop=True)
                    E = e_pool.tile([SROWS, jw], BF16, tag=f"E{jw}")
                    col = g * NSET + s
                    nc.scalar.activation(
                        E[:], l1[:], AF.Exp, scale=-1.0,
                        bias=biasn[:, col:col + 1],
                        accum_out=feat5[:, col:col + 1])
                    sc = selc_t[s if nr == GSZ else NSET + s]
                    nc.tensor.matmul(
                        cspsum[:, 0:jw - 128], sc[:], E[:, 128:jw],
                        start=(g == 0 and s == 0),
                        stop=(g == NG - 1 and s == NSET - 1))

        cs_sb = const.tile([K, CSW], FP32, tag="cssb")
        nc.vector.tensor_copy(cs_sb[:], cspsum[:])
        nc.sync.dma_start(colt_d[:, :], cs_sb[:])
        nc.sync.dma_start(feat5_d[:, :], feat5[:])

    _hoist_excess_waits(nc)
    return nc


_NC_CACHE = None


def _get_nc():
    global _NC_CACHE
    if _NC_CACHE is None:
        _NC_CACHE = build_nc()
    return _NC_CACHE


_SELR, _NEGC, _SELC = _sel_arrays()


def _in_maps(x, T):
    selr = np.ascontiguousarray(_SELR.reshape(GSZ * SROWS, SROWS))
    negc = np.ascontiguousarray(_NEGC.reshape(2 * SETK, SROWS))
    selc = np.ascontiguousarray(_SELC.reshape(2 * NSET * SROWS, K))
    maps = []
    for c in range(NCORES):
        xr = np.roll(x, -RPC * c, axis=0)
        tc_ = T if c < NCORES // 2 else np.roll(T, -KD // 2, axis=1)
        t5 = tc_.reshape(F, K, D).sum(axis=2)
        maps.append({
            "xt": np.ascontiguousarray(xr.T).astype(NPBF),
            "t": np.ascontiguousarray(tc_).astype(NPBF),
            "t5": np.ascontiguousarray(t5).astype(NPBF),
            "selr": selr,
            "negc": negc,
            "selc": selc,
        })
    return maps


def _assemble(x, results):
    """Gather per-core feat5/colt outputs into the full [B, F+K] output."""
    feat = np.zeros((B, K), np.float32)
    kidx = np.arange(K)
    for c in range(NCORES):
        f5 = np.asarray(results[c]["feat5"], np.float32)
        ct = np.asarray(results[c]["colt"], np.float32)
        # f5[25r + kk, 4g + s] -> win[i_loc = 5g + r, k_loc = 25s + kk]
        win = (f5.reshape(GSZ, SETK, NG, NSET)
               .transpose(2, 0, 3, 1).reshape(NG * GSZ, K)[:RPC])
        kmap = (kidx + (K // 2 if c >= NCORES // 2 else 0)) % K
        rows = np.arange(RPC * c, RPC * c + RPC)
        feat[np.ix_(rows, kmap)] += win
        jcols = (RPC * c + 128 + np.arange(CSW)) % B
        feat[np.ix_(jcols, kmap[:K // 2])] += ct[:K // 2, :].T
        feat[np.ix_(jcols[:384], kmap[K // 2:])] += ct[K // 2:, :384].T
    return np.concatenate([x, feat], axis=1)


def kernel(x: np.ndarray, T: np.ndarray) -> np.ndarray:
    x = np.ascontiguousarray(np.asarray(x, dtype=np.float32))
    T = np.ascontiguousarray(np.asarray(T, dtype=np.float32))
    assert x.shape == (B, F) and T.shape == (F, KD)
    nc = _get_nc()
    res = run_bass_kernel_spmd(nc, _in_maps(x, T), list(range(NCORES)))
    return _assemble(x, res.results)
